# revision 11
# baseline (speedup 1.0000x reference)
"""v9: v6 + fast-start. The first tile's load is split into two
half-partition DMAs issued simultaneously on both HWDGE rings, so all 16
DMA engines see descriptors ~2x sooner and tile 0 finishes loading early
(both rings contribute), pulling the whole copy->store pipeline forward.
"""

import numpy as np

_B, _C, _H, _W = 32, 3, 512, 512
_N_CORES = 8
_ROWS = (_B // _N_CORES) * _C * (_H // 8)  # 768
_COLS = 8 * _W                             # 4096
_N_TILES = _ROWS // 128                    # 6

_nc_cache = None


def _build():
    import concourse.mybir as mybir
    from concourse import bacc

    nc = bacc.Bacc(
        "TRN2", target_bir_lowering=False, debug=False, num_devices=_N_CORES
    )
    x = nc.dram_tensor(
        "x", (_ROWS, _COLS), mybir.dt.float32, kind="ExternalInput"
    ).ap()
    y = nc.dram_tensor(
        "y", (_ROWS, _COLS), mybir.dt.float32, kind="ExternalOutput"
    ).ap()

    f32 = mybir.dt.float32
    with (
        nc.sbuf_tensor([128, _N_TILES * _COLS], f32) as tin,
        nc.sbuf_tensor([128, _N_TILES * _COLS], f32) as tout,
        nc.semaphore() as sem_ld_sp,   # loads on sync (SP ring)
        nc.semaphore() as sem_ld_act,  # loads on scalar (ACT ring)
        nc.semaphore() as sem_cp,      # vector copies
        nc.semaphore() as sem_st,      # stores, both rings
    ):
        # Tile 0 split by partition halves, one half per ring, issued first:
        # half-size descriptor sets generate ~2x faster and cover all 16
        # engines between them.
        nc.sync.dma_start(
            out=tin[0:64, 0:_COLS],
            in_=x[0:64, :],
            single_packet=True,
        ).then_inc(sem_ld_sp, 16)
        nc.scalar.dma_start(
            out=tin[64:128, 0:_COLS],
            in_=x[64:128, :],
            single_packet=True,
        ).then_inc(sem_ld_act, 16)

        # Remaining 5 full-tile loads alternate rings: L1,L3,L5 -> sync,
        # L2,L4 -> scalar.
        for t in range(1, _N_TILES):
            eng = nc.sync if t % 2 == 1 else nc.scalar
            sem = sem_ld_sp if t % 2 == 1 else sem_ld_act
            eng.dma_start(
                out=tin[:, t * _COLS:(t + 1) * _COLS],
                in_=x[t * 128:(t + 1) * 128, :],
                single_packet=True,
            ).then_inc(sem, 16)

        # Ring-local load-completion counts:
        #   sync:   L0a=16, L1=32, L3=48, L5=64
        #   scalar: L0b=16, L2=32, L4=48
        _ld_wait = {
            0: [(0, 16), (1, 16)],  # both halves
            1: [(0, 32)],
            2: [(1, 32)],
            3: [(0, 48)],
            4: [(1, 48)],
            5: [(0, 64)],
        }
        sems = (sem_ld_sp, sem_ld_act)
        for t in range(_N_TILES):
            for ring, val in _ld_wait[t]:
                nc.vector.wait_ge(sems[ring], val)
            src = tin[:, t * _COLS:(t + 1) * _COLS].rearrange(
                "p (r bw c) -> p bw r c", r=8, bw=64, c=8
            )
            dst = tout[:, t * _COLS:(t + 1) * _COLS].rearrange(
                "p (bw r c) -> p bw r c", bw=64, r=8, c=8
            )
            for s in range(2):
                bws = slice(s * 32, (s + 1) * 32)
                nc.vector.tensor_copy(out=dst[:, bws], in_=src[:, bws]).then_inc(
                    sem_cp, 1
                )

        # Stores: one full-tile store per tile, alternating rings.
        for t in range(_N_TILES):
            eng = nc.scalar if t % 2 == 0 else nc.sync
            eng.wait_ge(sem_cp, 2 * t + 2)
            eng.dma_start(
                out=y[t * 128:(t + 1) * 128, :],
                in_=tout[:, t * _COLS:(t + 1) * _COLS],
                single_packet=True,
            ).then_inc(sem_st, 16)

        nc.gpsimd.wait_ge(sem_st, 16 * _N_TILES)

        nc.compile()
    return nc


def kernel(x: np.ndarray) -> np.ndarray:
    from concourse import bass_utils

    global _nc_cache
    if _nc_cache is None:
        _nc_cache = _build()
    nc = _nc_cache

    x = np.ascontiguousarray(x, dtype=np.float32)
    assert x.shape == (_B, _C, _H, _W), x.shape
    xs = x.reshape(_N_CORES, _ROWS, _COLS)
    in_maps = [{"x": xs[k]} for k in range(_N_CORES)]
    res = bass_utils.run_bass_kernel_spmd(
        nc, in_maps, core_ids=list(range(_N_CORES))
    )
    ys = np.stack([res.results[k]["y"] for k in range(_N_CORES)], axis=0)
    return ys.reshape(_B, _C, 1, _H, _W)
